# revision 22
# baseline (speedup 1.0000x reference)
"""Trainium2 Bass kernel: spatial self-attention block (RMSNorm + QKV 1x1conv +
8-head attention over 32x32 positions + out-proj + residual).

Input x: [8, 512, 32, 32] f32. Data-parallel: one batch element per NeuronCore
(8 cores). No collectives.

Per-core math (x: [C=512, S=1024]):
  inv[s]  = sqrt(512) / ||x[:, s]||            (g folded into W)
  q,k     = Wq@xn, Wk@xn stored [d, s] per head-pair tiles; q pre-scaled
  vT      = xn^T @ Wv^T stored [s, d] (+ 1/inv column per head for denom)
  S^T_h   = k_h^T' q_h  -- two K=64 matmuls on distinct PE row groups run
            concurrently (tile_position row tiling), no zero padding
  E       = exp(S^T + ln inv_j): split across three engines --
            ACT exact LUT exp; DVE + GPSIMD use a Schraudolph bit-trick:
            bf16(exp(x)) == bitcast_bf16(int16(x*log2e*128 + B)), one
            tensor_scalar (mult, add -> int16) per tile
  [O^T_h; den_h] = [vT_h|rinv]^T' E   (M=65 matmul: row 64 = denominator)
  1/den via DVE reciprocal_approx_fast directly off PSUM row 64
  O^T_h  *= bcast(1/den)  (broadcast via csel matmul)
  out     = Wout^T' O_flat + b + x    (residual from bf16 x; f32 x not loaded)
"""

import numpy as np

import concourse.bass as bass
import concourse.bacc as bacc
import concourse.tile as tile
from concourse import mybir
from concourse.bass_utils import run_bass_kernel_spmd

F32 = mybir.dt.float32
F32R = mybir.dt.float32r
BF16 = mybir.dt.bfloat16
I16 = mybir.dt.int16

B = 8
C = 512          # channels
S = 1024         # spatial positions
H = 8            # heads
D = 64           # dim per head
HID = H * D      # 512
SCALE = D ** -0.5
NCT = C // 128   # 4 channel tiles
NP = H // 2      # 4 head pairs
NJT = S // 128   # 8 j-tiles
NOT = C // 128   # 4 output-channel tiles
NH2 = S // 512   # 2 free-dim halves

# Schraudolph exp -> bf16 bits: int16(x * LOG2E*128 + SCH_B16) viewed as bf16
LOG2E = 1.4426950408889634
SCH_A16 = LOG2E * 128.0
SCH_B16 = (127.0 * (1 << 23) - 545947.0) / 65536.0

# column offsets in the host-reordered wqkvT [C, 1536]
OFF_Q = [0, 768, 1024, 1280]
OFF_K = [128, 896, 1152, 1408]
OFF_V = 256

_CACHE = {}


def _r(ap):
    return ap.bitcast(F32R)


def _pin_act_tables():
    if getattr(bacc, "_act_tables_pinned", False):
        return
    orig = bacc.get_activation_tables

    def pinned(arch):
        tables = orig(arch)
        return {k: (v if k == "natural_log_exp_and_others" else set())
                for k, v in tables.items()}

    bacc.get_activation_tables = pinned
    bacc._act_tables_pinned = True


def _build_nc():
    _pin_act_tables()
    nc = bacc.Bacc()
    AF = mybir.ActivationFunctionType
    ALU = mybir.AluOpType

    xb_ext = nc.declare_dram_parameter("xb16", [C, S], BF16, isOutput=False)
    wq_ext = nc.declare_dram_parameter("wqkvT", [C, 3 * HID], BF16, isOutput=False)
    wo_ext = nc.declare_dram_parameter("woutT", [HID, C], BF16, isOutput=False)
    bout_ext = nc.declare_dram_parameter("bout", [C, 1], F32, isOutput=False)
    csel_ext = nc.declare_dram_parameter("cselbig", [128, 128], F32, isOutput=False)
    cones_ext = nc.declare_dram_parameter("cones", [128, 1], BF16, isOutput=False)
    conesr_ext = nc.declare_dram_parameter("conesr", [1, 128], F32R, isOutput=False)
    out_ext = nc.declare_dram_parameter("out", [C, S], F32, isOutput=True)

    from contextlib import ExitStack
    with tile.TileContext(nc) as tc, ExitStack() as est:
        pool = lambda name, bufs, **kw: est.enter_context(
            tc.tile_pool(name=name, bufs=bufs, **kw))
        sb_x = pool("sb_x", NCT)
        sb_w = pool("sb_w", NCT)
        sb_wo = pool("sb_wo", NP)
        sb_small = pool("sb_small", 1)
        sb_x2 = pool("sb_x2", 2)
        sb_qk = pool("sb_qk", 1)
        sb_vt = pool("sb_vt", NJT)
        sb_es = pool("sb_es", 4)
        sb_of = pool("sb_of", NP)
        sb_out = pool("sb_out", 2)
        ps_big = pool("ps_big", 2, space="PSUM")   # [128,1024] f32: qk/st/psout
        ps_o = pool("ps_o", 2, space="PSUM")       # [*,1024] f32: sumsq/invb/vt/o/ib

        # ---- const + input DMAs (consumption order) ----
        ones_col = sb_small.tile([128, 1], BF16, tag="onescol")
        nc.gpsimd.dma_start(ones_col[:], cones_ext[:, :])
        ones_row = sb_small.tile([1, 128], F32R, tag="onesrow")
        nc.gpsimd.dma_start(ones_row[:], conesr_ext[:, :])
        csel = sb_small.tile([128, 128], F32, tag="csel")
        nc.gpsimd.dma_start(csel[:], csel_ext[:, :])
        bt = sb_small.tile([128, NOT], F32, tag="bt")
        for ot in range(NOT):
            nc.gpsimd.dma_start(bt[:, ot:ot + 1], bout_ext[ot * 128:(ot + 1) * 128, :])
        xb = []
        for ct in range(NCT):
            t = sb_x.tile([128, S], BF16, tag="xb", name=f"xb{ct}")
            nc.sync.dma_start(t[:], xb_ext[ct * 128:(ct + 1) * 128, :])
            xb.append(t)
        # wq in consumption-ordered column chunks: [q0k0 | v | q1k1 | q2k2 | q3k3]
        wq = [sb_w.tile([128, 3 * HID], BF16, tag="wq", name=f"wq{ct}")
              for ct in range(NCT)]
        for lo, hi in ((0, 256), (256, 768), (768, 1024)):
            for ct in range(NCT):
                nc.sync.dma_start(wq[ct][:, lo:hi],
                                  wq_ext[ct * 128:(ct + 1) * 128, lo:hi])
        wo = []
        for p in range(NP):
            t = sb_wo.tile([128, C], BF16, tag="wo")
            nc.sync.dma_start(t[:], wo_ext[p * 128:(p + 1) * 128, :])
            wo.append(t)
        for lo, hi in ((1024, 1280), (1280, 1536)):
            for ct in range(NCT):
                nc.sync.dma_start(wq[ct][:, lo:hi],
                                  wq_ext[ct * 128:(ct + 1) * 128, lo:hi])

        # ---- RMSNorm stats ----
        sumsq = ps_o.tile([1, S], F32, tag="o", name="sumsq")
        for ct in range(NCT):
            x2 = sb_x2.tile([128, S], BF16, tag="x2")
            if ct % 2 == 0:
                nc.scalar.activation(x2[:], xb[ct][:], AF.Square)
            else:
                nc.vector.tensor_mul(x2[:], xb[ct][:], xb[ct][:])
            for nh in range(NH2):
                nc.tensor.matmul(
                    sumsq[:, nh * 512:(nh + 1) * 512],
                    lhsT=ones_col[:],
                    rhs=x2[:, nh * 512:(nh + 1) * 512],
                    start=(ct == 0), stop=(ct == NCT - 1),
                )
        lnv = sb_small.tile([1, S], F32, tag="lnv")
        nc.scalar.activation(lnv[:], sumsq[:], AF.Ln)
        bln = sb_small.tile([1, 1], F32, tag="bln")
        nc.vector.memset(bln[:], 0.5 * float(np.log(C)))
        inv = sb_small.tile([1, S], F32R, tag="inv")
        nc.scalar.activation(inv[:], lnv[:], AF.Exp, bias=bln[:], scale=-0.5)
        invb = ps_o.tile([128, S], F32, tag="o", name="invb")
        for nh in range(NH2):
            nc.tensor.matmul(
                invb[:, nh * 512:(nh + 1) * 512],
                lhsT=ones_row[:],
                rhs=inv[:, nh * 512:(nh + 1) * 512],
                start=True, stop=True,
            )
        invb_sb = sb_small.tile([128, S], F32, tag="invbsb")
        nc.vector.tensor_copy(invb_sb[:], invb[:])
        lninv = sb_small.tile([1, S], F32, tag="lninv")
        nc.scalar.activation(lninv[:], lnv[:], AF.Identity, bias=bln[:], scale=-0.5)
        rinv = sb_small.tile([1, S], F32, tag="rinv")
        nc.scalar.activation(rinv[:], lninv[:], AF.Exp, scale=-1.0)
        lninvt = sb_small.tile([128, NJT], F32, tag="lninvt")
        rinvt = sb_small.tile([128, NJT], F32, tag="rinvt")
        for jt in range(NJT):
            eng = nc.gpsimd if jt % 2 == 0 else nc.sync
            eng.dma_start(lninvt[:, jt:jt + 1], lninv[:, jt * 128:(jt + 1) * 128])
        for jt in range(NJT):
            eng = nc.gpsimd if jt % 2 == 0 else nc.sync
            eng.dma_start(rinvt[:, jt:jt + 1], rinv[:, jt * 128:(jt + 1) * 128])
        # Schraudolph per-partition bias: B2t = A16 * lninvt + B16
        b2t = sb_small.tile([128, NJT], F32, tag="b2t")
        nc.vector.tensor_scalar(b2t[:], lninvt[:], SCH_A16, SCH_B16,
                                op0=ALU.mult, op1=ALU.add)
        # 1/den staging: head h -> tile h//4, row 32*(h%4); rest stay 1.0 so a
        # full-tile reciprocal is safe
        dall = [sb_small.tile([128, S], F32, tag=f"dall{i}", name=f"dall{i}")
                for i in range(2)]
        for i in range(2):
            nc.gpsimd.memset(dall[i][:], 1.0)

        # ---- QKV projection; inv applied on evacuation ----
        qq = [None] * NP   # q pair tiles [128=2x64 d, S]
        kk = [None] * NP   # k pair tiles [128=2x64 d, S]

        def emit_qk(p, which):
            off = OFF_Q[p] if which == "q" else OFF_K[p]
            ps = ps_big.tile([128, S], F32, tag="big", name=f"qkps_{which}{p}")
            for nh in range(NH2):
                for ct in range(NCT):
                    nc.tensor.matmul(
                        ps[:, nh * 512:(nh + 1) * 512],
                        lhsT=wq[ct][:, off:off + 128],
                        rhs=xb[ct][:, nh * 512:(nh + 1) * 512],
                        start=(ct == 0), stop=(ct == NCT - 1),
                    )
            t = sb_qk.tile([128, S], BF16, tag=f"qk_{which}{p}",
                           name=f"qk_{which}{p}")
            nc.vector.tensor_mul(t[:], ps[:], invb_sb[:])
            if which == "q":
                qq[p] = t
            else:
                kk[p] = t

        emit_qk(0, "q")
        emit_qk(0, "k")

        # vT tiles: [s-tile 128, 8*65] = per head 64 v-dims + 1/inv column
        vt = []
        for jt in range(NJT):
            ps = ps_o.tile([128, HID], F32, tag="o", name=f"vtps{jt}")
            for ct in range(NCT):
                nc.tensor.matmul(
                    ps[:],
                    lhsT=xb[ct][:, jt * 128:(jt + 1) * 128],
                    rhs=wq[ct][:, OFF_V:OFF_V + HID],
                    start=(ct == 0), stop=(ct == NCT - 1),
                )
            t = sb_vt.tile([128, H * 65], BF16, tag="vt")
            t_v = t[:].rearrange("p (h e) -> p h e", e=65)
            nc.vector.tensor_copy(
                t_v[:, :, 64:65],
                rinvt[:, jt:jt + 1].to_broadcast((128, H, 1)))
            nc.scalar.copy(t_v[:, :, 0:64], ps[:].rearrange("p (h d) -> p h d", d=64))
            vt.append(t)

        for p in range(1, NP):
            emit_qk(p, "q")
            emit_qk(p, "k")

        # ---- attention (row-tiled S^T; 3-engine exp split) ----
        of = []
        o_live = []   # (p, [o_a, o_b]) awaiting normalize
        ib_done = []

        def emit_normalize(p, o_ab):
            ko = 64 * (p % 2)
            ib = ps_o.tile([128, S], F32, tag="o", name=f"ib{p}")
            for nh in range(NH2):
                nc.tensor.matmul(
                    ib[:, nh * 512:(nh + 1) * 512],
                    lhsT=csel[ko:ko + 64, :],
                    rhs=dall[p // 2][ko:ko + 64, nh * 512:(nh + 1) * 512],
                    start=True, stop=True,
                )
            nc.vector.tensor_mul(of[p][:], of[p][:], ib[:])
            ib_done.append(p)

        exp_rr = 0
        for p in range(NP):
            qp, kp = qq[p], kk[p]
            o_ab = [ps_o.tile([65, S], F32, tag="o", name=f"o{p}_{i}")
                    for i in range(2)]
            for jt in range(NJT):
                sts = []
                for half in range(2):
                    lo = 64 * half
                    st = ps_big.tile([128, S], F32, tag="big",
                                     name=f"st{p}_{jt}_{half}")
                    sts.append(st)
                    for nh in range(NH2):
                        nc.tensor.matmul(
                            st[:, nh * 512:(nh + 1) * 512],
                            lhsT=kp[lo:lo + 64, jt * 128:(jt + 1) * 128],
                            rhs=qp[lo:lo + 64, nh * 512:(nh + 1) * 512],
                            start=True, stop=True,
                        )
                ess = []
                for half in range(2):
                    es = sb_es.tile([128, S], BF16, tag="es",
                                    name=f"es{p}_{jt}_{half}")
                    ess.append(es)
                    # ACT exact exp for 40/64 tiles; DVE Schraudolph for 24/64
                    if half == 0 or jt % 4 == 3:
                        nc.scalar.activation(es[:], sts[half][:], AF.Exp,
                                             bias=lninvt[:, jt:jt + 1])
                    else:
                        nc.vector.tensor_scalar(
                            es[:].bitcast(I16), sts[half][:],
                            SCH_A16, b2t[:, jt:jt + 1],
                            op0=ALU.mult, op1=ALU.add)
                for half in range(2):
                    h = 2 * p + half
                    for nh in range(NH2):
                        nc.tensor.matmul(
                            o_ab[half][:, nh * 512:(nh + 1) * 512],
                            lhsT=vt[jt][:, h * 65:(h + 1) * 65],
                            rhs=ess[half][:, nh * 512:(nh + 1) * 512],
                            start=(jt == 0), stop=(jt == NJT - 1),
                        )
            # denominators -> dall[p//2] rows 32*(2p%4), 32*(2p%4+1); one
            # in-place full-tile reciprocal per dall tile (idle rows are 1.0)
            for half in range(2):
                h = 2 * p + half
                row = 32 * (h % 4)
                if half == 0:
                    nc.vector.tensor_copy(dall[h // 4][row:row + 1, :],
                                          o_ab[half][64:65, :])
                else:
                    nc.scalar.copy(dall[h // 4][row:row + 1, :],
                                   o_ab[half][64:65, :])
            if p % 2 == 1:
                nc.vector.reciprocal_approx_fast(
                    out=dall[p // 2][:], in_=dall[p // 2][:])
            t = sb_of.tile([128, S], BF16, tag="of")
            for half in range(2):
                nc.vector.tensor_copy(t[64 * half:64 * (half + 1), :],
                                      o_ab[half][0:64, :])
            of.append(t)
            o_live.append((p, o_ab))
            # normalize pairs whose ib can now take a freed o slot
            if p >= 1 and len(o_live) > 2:
                pn, _ = o_live.pop(0)
                emit_normalize(pn, None)

        for pn in range(NP):
            if pn not in ib_done:
                emit_normalize(pn, None)

        # ---- output projection + bias + residual (bf16 x) ----
        for ot in range(NOT):
            ps = ps_big.tile([128, S], F32, tag="big", name=f"psout{ot}")
            for nh in range(NH2):
                for p in range(NP):
                    nc.tensor.matmul(
                        ps[:, nh * 512:(nh + 1) * 512],
                        lhsT=wo[p][:, ot * 128:(ot + 1) * 128],
                        rhs=of[p][:, nh * 512:(nh + 1) * 512],
                        start=(p == 0), stop=(p == NP - 1),
                    )
            t = sb_out.tile([128, S], F32, tag="outt")
            for nh in range(NH2):
                sl = slice(nh * 512, (nh + 1) * 512)
                eng = nc.vector
                eng.scalar_tensor_tensor(
                    t[:, sl], ps[:, sl], bt[:, ot:ot + 1], xb[ot][:, sl],
                    op0=mybir.AluOpType.add, op1=mybir.AluOpType.add,
                )
            nc.sync.dma_start(out_ext[ot * 128:(ot + 1) * 128, :], t[:])

    nc.finalize()
    return nc


def _prep_consts():
    csel = np.zeros((128, 128), np.float32)
    csel[0, 0:64] = 1.0
    csel[32, 64:128] = 1.0
    csel[64, 0:64] = 1.0
    csel[96, 64:128] = 1.0
    import ml_dtypes
    cones = np.ones((128, 1), ml_dtypes.bfloat16)
    conesr = np.ones((1, 128), np.float32)
    return csel, cones, conesr


def _prep_weights(w_qkv, w_out, b_out, g):
    gc = np.asarray(g, np.float32).reshape(C)
    w_eff = np.asarray(w_qkv, np.float32) * gc[None, :]
    w_eff = w_eff.copy()
    w_eff[:HID] *= SCALE                    # fold q scaling
    # reorder rows -> [q0 k0 v q1 k1 q2 k2 q3 k3] then transpose to [C, 1536]
    blocks = [w_eff[0:128], w_eff[HID:HID + 128], w_eff[2 * HID:3 * HID]]
    for p in range(1, NP):
        blocks.append(w_eff[p * 128:(p + 1) * 128])
        blocks.append(w_eff[HID + p * 128:HID + (p + 1) * 128])
    w_re = np.concatenate(blocks, axis=0)
    import ml_dtypes
    wqkvT = np.ascontiguousarray(w_re.T).astype(ml_dtypes.bfloat16)   # [C, 1536]
    woutT = np.ascontiguousarray(
        np.asarray(w_out, np.float32).T).astype(ml_dtypes.bfloat16)    # [HID, C]
    bout = np.asarray(b_out, np.float32).reshape(C, 1)
    return wqkvT, woutT, bout


def _get_nc():
    if "nc" not in _CACHE:
        _CACHE["nc"] = _build_nc()
    return _CACHE["nc"]


def run(inputs, trace=False, trace_cores=None):
    x = np.asarray(inputs["x"], np.float32)
    wqkvT, woutT, bout = _prep_weights(
        inputs["w_qkv"], inputs["w_out"], inputs["b_out"], inputs["g"])
    csel, cones, conesr = _prep_consts()

    in_maps = []
    for b in range(B):
        import ml_dtypes
        xc = np.ascontiguousarray(x[b].reshape(C, S))
        in_maps.append({
            "xb16": xc.astype(ml_dtypes.bfloat16),
            "wqkvT": wqkvT,
            "woutT": woutT,
            "bout": bout,
            "cselbig": csel,
            "cones": cones,
            "conesr": conesr,
        })

    nc = _get_nc()
    res = run_bass_kernel_spmd(
        nc, in_maps, core_ids=list(range(B)),
        trace=trace, trace_cores=trace_cores,
    )
    out = np.stack([res.results[b]["out"].reshape(C, 32, 32) for b in range(B)])
    return out.astype(np.float32), res


def kernel(**inputs):
    out, _ = run(inputs, trace=False)
    return out
